# revision 7
# baseline (speedup 1.0000x reference)
"""Trainium2 Bass kernel for nn_Attention_29661044146348.

Diffusion-style attention block: GroupNorm(32) -> 1x1-conv qkv -> single-head
attention over h*w positions (d = C = 512) -> 1x1-conv out -> residual.
Input x is [8, 512, 64, 64]; batch is data-parallel across the 8 NeuronCores
(one batch element per core), no collectives.

Per-core layout strategy ("S^T layout" flash attention, zero transposes in the
hot path):
  - scores are computed transposed, S^T[j, i] (keys on partitions), via
    lhsT = K^T chunks, rhs = Q^T block -- both natural outputs of the qkv
    matmul.
  - P = exp(scale * S^T) (no max subtraction: scores are ~N(0,1) by GroupNorm
    + fan-in init, so fp32 exp cannot overflow); softmax denominators are
    accumulated on the Vector engine and reduced across partitions with a
    ones-vector matmul.
  - O^T = sum_j V[j-chunk] (as weights) @ P[j-chunk], which directly yields
    the [c, i] layout the output projection needs.
  - big matmuls run in float32r (TF32-like, 1 cycle/row on the PE for
    free-dim >= 256, ~4x fp32 rate; ~1e-3 relative error).
  - qkv/out weights are transposed on-chip with PE-transpose once.
  - v-bias is folded into an effective output bias (att rows sum to 1).
"""

import jax
import numpy as np
from jax.experimental.shard_map import shard_map
from jax.sharding import Mesh, NamedSharding, PartitionSpec

import bass_rust
import concourse.bass as bass
import concourse.tile as tile
from concourse import bass2jax, mybir
from concourse.masks import make_identity

F32 = mybir.dt.float32
F32R = mybir.dt.float32r

C = 512          # channels == attention dim
NT = C // 128    # channel tiles (4)
GROUPS = 32
EPS = 1e-5
ATT_SCALE = float(C) ** -0.5
IB = 256         # attention i-block (queries per block)


def _split_multi_waits(nc):
    """The staged walrus build rejects >1 sync-wait per instruction; hoist
    extra waits onto single-wait NOPs placed immediately before."""
    ctr = 0
    for bb in nc.main_func.blocks:
        insts = bb.instructions
        i = 0
        while i < len(insts):
            ins = insts[i]
            si = ins.sync_info
            if si is not None:
                waits = list(si.on_wait)
                if len(waits) > 1:
                    si.on_wait = waits[-1:]
                    for w in waits[:-1]:
                        nop = mybir.InstNoOp(name=f"waitsplit-{ctr}", ins=[], outs=[])
                        ctr += 1
                        nop.engine = ins.engine
                        nop.sync_info = bass_rust.SyncInfo(on_wait=[w], on_update=[])
                        nc.register_instruction(nop, overwrite=True)
                        insts.insert(i, nop)
                        i += 1
            i += 1
    return ctr


def build_nc(S):
    NSUB = S // 512   # bn_stats subgroups per channel row
    S8 = S // 512     # qkv seq chunks
    JT = S // 128     # attention key chunks
    NIB = S // IB     # attention query blocks

    nc = bass.Bass()
    x_ext = nc.declare_dram_parameter("x", [C, S], F32, isOutput=False)
    gnw_ext = nc.declare_dram_parameter("gn_weight", [C], F32, isOutput=False)
    gnb_ext = nc.declare_dram_parameter("gn_bias", [C], F32, isOutput=False)
    qkvw_ext = nc.declare_dram_parameter("qkv_w", [3 * C, C], F32, isOutput=False)
    qkvb_ext = nc.declare_dram_parameter("qkv_b", [3 * C], F32, isOutput=False)
    outw_ext = nc.declare_dram_parameter("out_w", [C, C], F32, isOutput=False)
    outb_ext = nc.declare_dram_parameter("out_b", [C], F32, isOutput=False)
    out_ext = nc.declare_dram_parameter("out", [C, S], F32, isOutput=True)
    qspill = nc.dram_tensor("q_spill", [C, S], F32R)

    xv = x_ext[:].rearrange("(t p) s -> p t s", p=128)
    qsv = qspill[:].rearrange("(t p) s -> p t s", p=128)
    ov = out_ext[:].rearrange("(t p) s -> p t s", p=128)

    with tile.TileContext(nc) as tc:
        with (
            tc.tile_pool(name="consts", bufs=1) as consts,
            tc.tile_pool(name="big", bufs=1) as big,
            tc.tile_pool(name="gn_small", bufs=1) as gn_small,
        ):
            # ---------------- constants ----------------
            ident = consts.tile([128, 128], F32)
            make_identity(nc, ident)
            ones128 = consts.tile([128, 1], F32)
            nc.vector.memset(ones128, 1.0)
            ones1 = consts.tile([1, 128], F32)
            nc.vector.memset(ones1, 1.0)
            ind = consts.tile([128, 8], F32)       # ind[p,g] = (p//16 == g)
            nc.vector.memset(ind, 1.0)
            nc.gpsimd.affine_select(
                out=ind, in_=ind, compare_op=mybir.AluOpType.is_ge, fill=0.0,
                base=0, pattern=[[-16, 8]], channel_multiplier=1)
            nc.gpsimd.affine_select(
                out=ind, in_=ind, compare_op=mybir.AluOpType.is_ge, fill=0.0,
                base=15, pattern=[[16, 8]], channel_multiplier=-1)
            indT = consts.tile([8, 128], F32)
            nc.vector.memset(indT, 1.0)
            nc.gpsimd.affine_select(
                out=indT, in_=indT, compare_op=mybir.AluOpType.is_ge, fill=0.0,
                base=0, pattern=[[1, 128]], channel_multiplier=-16)
            nc.gpsimd.affine_select(
                out=indT, in_=indT, compare_op=mybir.AluOpType.is_ge, fill=0.0,
                base=15, pattern=[[-1, 128]], channel_multiplier=16)
            eps8 = consts.tile([8, 1], F32)
            nc.vector.memset(eps8, EPS)

            wv = consts.tile([128, NT], F32)
            nc.sync.dma_start(out=wv[:], in_=gnw_ext[:].rearrange("(t p) -> p t", p=128))
            bv = consts.tile([128, NT], F32)
            nc.sync.dma_start(out=bv[:], in_=gnb_ext[:].rearrange("(t p) -> p t", p=128))
            qb = consts.tile([128, NT], F32)
            nc.sync.dma_start(out=qb[:], in_=qkvb_ext[0:C].rearrange("(t p) -> p t", p=128))
            kb = consts.tile([128, NT], F32)
            nc.sync.dma_start(out=kb[:], in_=qkvb_ext[C:2 * C].rearrange("(t p) -> p t", p=128))
            vb = consts.tile([128, NT], F32)
            nc.sync.dma_start(out=vb[:], in_=qkvb_ext[2 * C:3 * C].rearrange("(t p) -> p t", p=128))
            obt = consts.tile([128, NT], F32)
            nc.sync.dma_start(out=obt[:], in_=outb_ext[:].rearrange("(t p) -> p t", p=128))

            # ---------------- GroupNorm statistics ----------------
            stats8 = gn_small.tile([128, 2, NT], F32)  # per-channel mean, E[x^2]
            with (
                tc.tile_pool(name="xload", bufs=2) as xload,
                tc.tile_pool(name="stp", bufs=2) as stp,
                tc.tile_pool(name="psg", bufs=1, space="PSUM") as psg,
            ):
                for t in range(NT):
                    xs = xload.tile([128, S], F32)
                    nc.sync.dma_start(out=xs[:], in_=x_ext[t * 128:(t + 1) * 128, :])
                    st = stp.tile([128, NSUB, 6], F32)
                    for sub in range(NSUB):
                        nc.vector.bn_stats(out=st[:, sub, :], in_=xs[:, sub * 512:(sub + 1) * 512])
                    mvt = stp.tile([128, 2], F32)
                    nc.vector.bn_aggr(out=mvt[:], in_=st[:])
                    nc.vector.tensor_copy(stats8[:, 0, t:t + 1], mvt[:, 0:1])
                    sqt = stp.tile([128, 1], F32)
                    nc.vector.tensor_mul(sqt[:], mvt[:, 0:1], mvt[:, 0:1])
                    nc.vector.tensor_add(stats8[:, 1, t:t + 1], mvt[:, 1:2], sqt[:])

                psG = psg.tile([8, 2, NT], F32)
                nc.tensor.matmul(psG[:], ind[:], stats8[:], start=True, stop=True)
                gsb = gn_small.tile([8, 2, NT], F32)
                nc.vector.tensor_scalar_mul(gsb[:], psG[:], 1.0 / 16.0)
                sq8 = gn_small.tile([8, NT], F32)
                nc.vector.tensor_mul(sq8[:], gsb[:, 0, :], gsb[:, 0, :])
                varr = gn_small.tile([8, NT], F32)
                nc.vector.tensor_sub(varr[:], gsb[:, 1, :], sq8[:])
                sd8 = gn_small.tile([8, NT], F32)
                nc.scalar.activation(out=sd8[:], in_=varr[:],
                                     func=mybir.ActivationFunctionType.Sqrt,
                                     bias=eps8[:], scale=1.0)
                nc.vector.reciprocal(gsb[:, 1, :], sd8[:])
                psBC = psg.tile([128, 2, NT], F32)
                nc.tensor.matmul(psBC[:], indT[:], gsb[:], start=True, stop=True)
                chst = gn_small.tile([128, 2, NT], F32)
                nc.vector.tensor_copy(chst[:], psBC[:])
                gA = gn_small.tile([128, NT], F32)
                nc.vector.tensor_mul(gA[:], chst[:, 1, :], wv[:])
                tmp4 = gn_small.tile([128, NT], F32)
                nc.vector.tensor_mul(tmp4[:], chst[:, 0, :], gA[:])
                gB = gn_small.tile([128, NT], F32)
                nc.vector.tensor_sub(gB[:], bv[:], tmp4[:])

            # ---------------- weight transposes ----------------
            # wT[p, t, o] = qkv_w[o, t*128+p]; owT[p, t, o] = out_w[o, t*128+p]
            wTp_cm = tc.tile_pool(name="wTp", bufs=1)
            wTp = wTp_cm.__enter__()
            wT = wTp.tile([128, NT, 3 * C], F32R)
            owT = big.tile([128, NT, C], F32R)
            ob_eff = consts.tile([128, NT], F32)
            with (
                tc.tile_pool(name="wnat", bufs=3) as wnat,
                tc.tile_pool(name="pst", bufs=3, space="PSUM") as pst,
            ):
                for r in range(3 * C // 128):
                    wn = wnat.tile([128, C], F32)
                    nc.sync.dma_start(out=wn[:], in_=qkvw_ext[r * 128:(r + 1) * 128, :])
                    for c4 in range(NT):
                        psT = pst.tile([128, 128], F32)
                        nc.tensor.transpose(psT[:], wn[:, c4 * 128:(c4 + 1) * 128], ident[:])
                        nc.vector.tensor_copy(wT[:, c4, r * 128:(r + 1) * 128], psT[:])
                for r in range(C // 128):
                    wn = wnat.tile([128, C], F32)
                    nc.sync.dma_start(out=wn[:], in_=outw_ext[r * 128:(r + 1) * 128, :])
                    for c4 in range(NT):
                        psT = pst.tile([128, 128], F32)
                        nc.tensor.transpose(psT[:], wn[:, c4 * 128:(c4 + 1) * 128], ident[:])
                        nc.vector.tensor_copy(owT[:, c4, r * 128:(r + 1) * 128], psT[:])

                # effective out bias: out_b + out_w @ v_bias (since att rows sum to 1)
                # fp32r matmuls need an even free dim, so pad the rhs to N=2
                vb2 = wnat.tile([128, NT, 2], F32)
                nc.vector.memset(vb2[:], 0.0)
                for c4 in range(NT):
                    nc.vector.tensor_copy(vb2[:, c4, 0:1], vb[:, c4:c4 + 1])
                vbr = wnat.tile([128, NT, 2], F32R)
                nc.vector.tensor_copy(vbr[:], vb2[:])
                for oc in range(NT):
                    psE = pst.tile([128, 2], F32, tag="psE")
                    for c4 in range(NT):
                        nc.tensor.matmul(psE[:], owT[:, c4, oc * 128:(oc + 1) * 128],
                                         vbr[:, c4, :],
                                         start=(c4 == 0), stop=(c4 == NT - 1))
                    nc.vector.tensor_add(ob_eff[:, oc:oc + 1], psE[:, 0:1], obt[:, oc:oc + 1])

            # ---------------- qkv projection (streamed over seq) ----------------
            kT = big.tile([128, NT, S], F32R)    # K^T  [c, s]
            Vt = big.tile([128, JT, C], F32R)    # V    [s, c] by key chunk
            with (
                tc.tile_pool(name="xc32", bufs=2) as xc32p,
                tc.tile_pool(name="xcr", bufs=2) as xcrp,
                tc.tile_pool(name="qsb", bufs=1) as qsbp,
                tc.tile_pool(name="psq", bufs=4, space="PSUM") as psq,
            ):
                for s8 in range(S8):
                    sl = slice(s8 * 512, (s8 + 1) * 512)
                    xc32 = xc32p.tile([128, NT, 512], F32)
                    nc.sync.dma_start(out=xc32[:], in_=xv[:, :, sl])
                    xcr = xcrp.tile([128, NT, 512], F32R)
                    for t in range(NT):
                        nc.vector.tensor_scalar(
                            out=xcr[:, t, :], in0=xc32[:, t, :],
                            scalar1=gA[:, t:t + 1], scalar2=gB[:, t:t + 1],
                            op0=mybir.AluOpType.mult, op1=mybir.AluOpType.add)
                    # K^T
                    for o4 in range(NT):
                        psK = psq.tile([128, 512], F32, tag="psq")
                        for c4 in range(NT):
                            nc.tensor.matmul(psK[:], wT[:, c4, C + o4 * 128:C + (o4 + 1) * 128],
                                             xcr[:, c4, :],
                                             start=(c4 == 0), stop=(c4 == NT - 1))
                        nc.scalar.activation(out=kT[:, o4, sl], in_=psK[:],
                                             func=mybir.ActivationFunctionType.Identity,
                                             bias=kb[:, o4:o4 + 1], scale=1.0)
                    # Q^T -> spill to DRAM
                    qsb = qsbp.tile([128, NT, 512], F32R)
                    for o4 in range(NT):
                        psQ = psq.tile([128, 512], F32, tag="psq")
                        for c4 in range(NT):
                            nc.tensor.matmul(psQ[:], wT[:, c4, o4 * 128:(o4 + 1) * 128],
                                             xcr[:, c4, :],
                                             start=(c4 == 0), stop=(c4 == NT - 1))
                        nc.scalar.activation(out=qsb[:, o4, :], in_=psQ[:],
                                             func=mybir.ActivationFunctionType.Identity,
                                             bias=qb[:, o4:o4 + 1], scale=1.0)
                    nc.sync.dma_start(out=qsv[:, :, sl], in_=qsb[:])
                    # V (keys on partitions): lhsT = xn chunk, rhs = w_v^T
                    for j4 in range(4):
                        psV = psq.tile([128, 512], F32, tag="psq")
                        for c4 in range(NT):
                            nc.tensor.matmul(psV[:], xcr[:, c4, j4 * 128:(j4 + 1) * 128],
                                             wT[:, c4, 2 * C:3 * C],
                                             start=(c4 == 0), stop=(c4 == NT - 1))
                        nc.vector.tensor_copy(Vt[:, s8 * 4 + j4, :], psV[:])

            wTp_cm.__exit__(None, None, None)

            # ---------------- attention + output projection ----------------
            with (
                tc.tile_pool(name="qblk", bufs=1) as qblkp,
                tc.tile_pool(name="xres", bufs=1) as xresp,
                tc.tile_pool(name="Pp", bufs=1) as Pp,
                tc.tile_pool(name="accp", bufs=2) as accp,
                tc.tile_pool(name="rsp", bufs=2) as rsp,
                tc.tile_pool(name="rbcp", bufs=2) as rbcp,
                tc.tile_pool(name="oTp", bufs=2) as oTp,
                tc.tile_pool(name="osbp", bufs=2) as osbp,
                tc.tile_pool(name="psS", bufs=3, space="PSUM") as psSp,
                tc.tile_pool(name="psO", bufs=3, space="PSUM") as psOp,
                tc.tile_pool(name="psM", bufs=1, space="PSUM") as psMp,
            ):
                for n in range(NIB):
                    il = slice(n * IB, (n + 1) * IB)
                    qblk = qblkp.tile([128, NT, IB], F32R)
                    nc.sync.dma_start(out=qblk[:], in_=qsv[:, :, il])
                    xres = xresp.tile([128, NT, IB], F32)
                    nc.sync.dma_start(out=xres[:], in_=xv[:, :, il])

                    P = Pp.tile([128, JT, IB], F32R)
                    acc = accp.tile([128, IB], F32)
                    for j in range(JT):
                        psS = psSp.tile([128, IB], F32, tag="psS")
                        for c4 in range(NT):
                            nc.tensor.matmul(psS[:], kT[:, c4, j * 128:(j + 1) * 128],
                                             qblk[:, c4, :],
                                             start=(c4 == 0), stop=(c4 == NT - 1))
                        nc.scalar.activation(out=P[:, j, :], in_=psS[:],
                                             func=mybir.ActivationFunctionType.Exp,
                                             scale=ATT_SCALE)
                        if j == 0:
                            nc.vector.tensor_copy(acc[:], P[:, 0, :].bitcast(F32))
                        else:
                            nc.vector.tensor_add(acc[:], acc[:], P[:, j, :].bitcast(F32))

                    # softmax denominators: cross-partition sum, reciprocal, broadcast
                    psR = psMp.tile([1, IB], F32, tag="psR")
                    nc.tensor.matmul(psR[:], ones128[:], acc[:], start=True, stop=True)
                    rs = rsp.tile([1, IB], F32)
                    nc.vector.reciprocal(rs[:], psR[:])
                    psB2 = psMp.tile([128, IB], F32, tag="psB2")
                    nc.tensor.matmul(psB2[:], ones1[:], rs[:], start=True, stop=True)
                    rbc = rbcp.tile([128, IB], F32)
                    nc.vector.tensor_copy(rbc[:], psB2[:])

                    # O^T = sum_j V[j] (weights) @ P[j], normalized on drain
                    oT = oTp.tile([128, NT, IB], F32R)
                    for c4 in range(NT):
                        psO = psOp.tile([128, IB], F32, tag="psO")
                        for j in range(JT):
                            nc.tensor.matmul(psO[:], Vt[:, j, c4 * 128:(c4 + 1) * 128],
                                             P[:, j, :],
                                             start=(j == 0), stop=(j == JT - 1))
                        nc.vector.tensor_mul(oT[:, c4, :], psO[:], rbc[:])

                    # output projection + bias + residual
                    osb = osbp.tile([128, NT, IB], F32)
                    for oc in range(NT):
                        psU = psOp.tile([128, IB], F32, tag="psO")
                        for c4 in range(NT):
                            nc.tensor.matmul(psU[:], owT[:, c4, oc * 128:(oc + 1) * 128],
                                             oT[:, c4, :],
                                             start=(c4 == 0), stop=(c4 == NT - 1))
                        nc.scalar.activation(out=osb[:, oc, :], in_=psU[:],
                                             func=mybir.ActivationFunctionType.Identity,
                                             bias=ob_eff[:, oc:oc + 1], scale=1.0)
                        nc.vector.tensor_add(osb[:, oc, :], osb[:, oc, :], xres[:, oc, :])
                    nc.sync.dma_start(out=ov[:, :, il], in_=osb[:])

    _split_multi_waits(nc)
    return nc


_RUNNER_CACHE = {}


class _Runner:
    """Builds the Bass graph once, compiles it through PJRT (shard_map over
    the 8 axon NeuronCores), and allows repeated execution for timing."""

    def __init__(self, S):
        self.S = S
        self.nc = build_nc(S)
        bass2jax.install_neuronx_cc_hook()
        nc = self.nc
        partition_name = (
            nc.partition_id_tensor.name if nc.partition_id_tensor else None
        )
        in_names, out_names, out_avals, zero_outs = [], [], [], []
        for alloc in nc.m.functions[0].allocations:
            if not isinstance(alloc, mybir.MemoryLocationSet):
                continue
            name = alloc.memorylocations[0].name
            if alloc.kind == "ExternalInput":
                if name != partition_name:
                    in_names.append(name)
            elif alloc.kind == "ExternalOutput":
                out_names.append(name)
                shape = tuple(alloc.tensor_shape)
                dtype = mybir.dt.np(alloc.dtype)
                out_avals.append(jax.core.ShapedArray(shape, dtype))
                zero_outs.append(np.zeros(shape, dtype))
        self.in_names = list(in_names)
        self.out_names = out_names
        self.out_avals = out_avals
        self.zero_outs = zero_outs
        all_in_names = in_names + out_names
        if partition_name is not None:
            all_in_names = all_in_names + [partition_name]

        def _body(*args):
            operands = list(args)
            if partition_name is not None:
                operands.append(bass2jax.partition_id_tensor())
            outs = bass2jax._bass_exec_p.bind(
                *operands,
                out_avals=tuple(out_avals),
                in_names=tuple(all_in_names),
                out_names=tuple(out_names),
                lowering_input_output_aliases=(),
                sim_require_finite=True,
                sim_require_nnan=True,
                nc=nc,
            )
            return tuple(outs)

        devices = jax.devices()[:8]
        self.mesh = Mesh(np.asarray(devices), ("core",))
        n_in = len(in_names) + len(out_names)
        self._fn = jax.jit(
            shard_map(
                _body, mesh=self.mesh,
                in_specs=(PartitionSpec("core"),) * n_in,
                out_specs=(PartitionSpec("core"),) * len(out_names),
                check_rep=False,
            )
        )

    def prepare(self, in_maps):
        sharding = NamedSharding(self.mesh, PartitionSpec("core"))
        concat = []
        for name in self.in_names:
            concat.append(np.concatenate([np.asarray(m[name]) for m in in_maps], axis=0))
        for z in self.zero_outs:
            concat.append(np.zeros((8 * z.shape[0], *z.shape[1:]), z.dtype))
        return [jax.device_put(a, sharding) for a in concat]

    def run(self, dev_args):
        return self._fn(*dev_args)


def _get_runner(S):
    if S not in _RUNNER_CACHE:
        _RUNNER_CACHE[S] = _Runner(S)
    return _RUNNER_CACHE[S]


def make_in_maps(x, gn_weight, gn_bias, qkv_w, qkv_b, out_w, out_b):
    b, c, h, w = x.shape
    S = h * w
    in_maps = []
    shared = {
        "gn_weight": np.ascontiguousarray(gn_weight, dtype=np.float32),
        "gn_bias": np.ascontiguousarray(gn_bias, dtype=np.float32),
        "qkv_w": np.ascontiguousarray(qkv_w, dtype=np.float32),
        "qkv_b": np.ascontiguousarray(qkv_b, dtype=np.float32),
        "out_w": np.ascontiguousarray(out_w, dtype=np.float32),
        "out_b": np.ascontiguousarray(out_b, dtype=np.float32),
    }
    for i in range(b):
        m = dict(shared)
        m["x"] = np.ascontiguousarray(np.asarray(x)[i].reshape(c, S), dtype=np.float32)
        in_maps.append(m)
    return in_maps


def kernel(x, gn_weight, gn_bias, qkv_w, qkv_b, out_w, out_b):
    x = np.asarray(x)
    b, c, h, w = x.shape
    assert b == 8 and c == C
    S = h * w
    r = _get_runner(S)
    in_maps = make_in_maps(x, gn_weight, gn_bias, qkv_w, qkv_b, out_w, out_b)
    outs = r.run(r.prepare(in_maps))
    idx = r.out_names.index("out")
    arr = np.asarray(outs[idx]).reshape(b, c, h, w)
    return arr.astype(np.float32)
